# revision 34
# baseline (speedup 1.0000x reference)
"""Trainium2 Bass kernel for nn_DeltaEdgeModel (edge-attention GNN).

v2 strategy (8 NeuronCores, SPMD), derived from the v1 trace:
  - Shard the E=4096 query-edge dim: 512 q-edges/core. Replicate the
    x/K/V projections in bf16 (collectives cost ~15us + bytes/40GB/s,
    so gathering K/V would lose; PE has the headroom).
  - Scalar engine does ONLY softmax EXP (the wall: 8.4M exps/layer)
    plus ctx2 evacuation in the inter-layer window and the tail Gelu.
    PSUM evacuation otherwise lives on Vector (tensor_scalar_add for
    per-partition biases) and GpSimd (x/K/V full tiles). 1/denom via
    nc.vector.reciprocal (kills the Ln/Exp(-1) activation-table swaps).
  - All matmuls bf16 (v1 had fp32 Wq/identity/classifier matmuls at 4
    cycles/row). Residual stream stays exact fp32 via Vector adds of
    (psum + sbuf_f32); bn1 folds into the ef chunks host-side, so the
    identity-matmul residual-add trick is gone entirely.
  - Out-projection bias via a rank-1 matmul (bias_row x ones) into the
    same PSUM accumulation group.
  - Software-pipelined attention: emit QK(i) then attnV(i-1) so the PE
    never blocks on the exp->mask chain; projection blocks for the next
    k-window are interleaved between windows so PE idle gaps are filled
    and the first QK needs only x-block0/K-block0.
  - Layer-1 attention splits into two 256-query passes: pass A's output
    projection + bf16 AllGather overlap pass B. Layer 2 (no exchange)
    runs full-width; its ctx2/xloc2/q2 work fills the gather-B window,
    and K2/V2 blocks interleave into the layer-2 attention windows.
  - Softmax denominator from a 65th all-ones V column (attn rows sum to
    1 after normalize, so the V bias folds into the out-proj bias on
    host; bn2 folds into o1's bias so the gathered o1 carries it).
Host side does only data layout; all FLOPs on device.
"""

import sys
import os

for _p in ("/opt/trn_rl_repo", "/root/.axon_site/_ro/trn_rl_repo"):
    if os.path.isdir(_p) and _p not in sys.path:
        sys.path.insert(0, _p)

import numpy as np
import ml_dtypes

import concourse.bass as bass
import concourse.bacc as bacc
import concourse.mybir as mybir
import concourse.tile as tile
from concourse.bass_utils import run_bass_kernel_spmd

BF16 = ml_dtypes.bfloat16
F32 = mybir.dt.float32
BF = mybir.dt.bfloat16
AF = mybir.ActivationFunctionType

N_CORES = 8
N_NODES, E = 1024, 4096
D = 256          # edge dim
H = 4            # heads
HD = 64          # head dim
NCLS = 16
QL = E // N_CORES          # local query edges per core = 512
KT = E // 128              # k tiles = 32
SQ = 1.0 / np.sqrt(HD)     # folded into Wq/bq on host
QH = QL // 2               # q-split half = 256
DEBUG = False
import os as _os
F_BCAST = _os.environ.get("KV_BCAST", "1") == "1"   # stride-0 mask mul
F_MERGE = _os.environ.get("KV_MERGE", "1") == "1"   # 2-kt merged score tiles
F_RANK1 = _os.environ.get("KV_RANK1", "1") == "1"   # rank-1 bias matmul


def build_nc():
    nc = bacc.Bacc("TRN2", target_bir_lowering=False, debug=False,
                   num_devices=N_CORES)

    def din(name, shape, dt=F32):
        return nc.dram_tensor(name, shape, dt, kind="ExternalInput")

    # chunk-contiguous big activations (fewer DMA descriptors)
    g_ch_d = [din(f"g_ch{j}", [128, 2, 1024], BF) for j in range(4)]
    ef_ch_d = [din(f"ef_ch{j}", [128, 2, 1024], BF) for j in range(4)]  # +bn1
    g_loc = din("g_loc", [128, 2, QL], BF)
    ef_loc = din("ef_loc", [128, 2, QL], BF)        # bf16, bn1 folded in
    mask_d = [din(f"mask{j}", [128, 4, QL], BF) for j in range(8)]
    wn = [din(f"w_n{l}", [128, 2, D], BF) for l in (1, 2)]
    wq = [din(f"w_q{l}", [128, 2, D], BF) for l in (1, 2)]
    wk = [din(f"w_k{l}", [128, 2, D], BF) for l in (1, 2)]
    wv = [din(f"w_v{l}", [128, 2, D], BF) for l in (1, 2)]
    wo = [din(f"w_o{l}", [64, H, D], BF) for l in (1, 2)]
    bq = [din(f"b_q{l}", [128, 2]) for l in (1, 2)]
    bk = [din(f"b_k{l}", [128, 2]) for l in (1, 2)]
    bor = [din(f"b_or{l}", [1, 2, 128], BF) for l in (1, 2)]  # rank-1 rows
    bof = [din(f"b_of{l}", [128, 2]) for l in (1, 2)]         # column form
    wc1 = din("w_c1", [128, 2, D], BF)
    bc1 = din("b_c1", [128, 2])
    wc2 = din("w_c2", [128, 2, NCLS], BF)
    bc2 = din("b_c2", [NCLS, 1])
    id16 = din("id16", [NCLS, NCLS])

    out = nc.dram_tensor("out", [QL, NCLS], F32, kind="ExternalOutput")
    dbg = {}
    if DEBUG:
        dbg["x_t"] = nc.dram_tensor("dbg_x_t", [128, 2, E], BF, kind="ExternalOutput")
        dbg["k_t"] = nc.dram_tensor("dbg_k_t", [128, 2, E], BF, kind="ExternalOutput")
        dbg["q_t"] = nc.dram_tensor("dbg_q_t", [128, 2, QL], BF, kind="ExternalOutput")
        dbg["o1"] = nc.dram_tensor("dbg_o1", [128, 2, QL], F32, kind="ExternalOutput")
        dbg["x2"] = nc.dram_tensor("dbg_x2", [128, 2, E], BF, kind="ExternalOutput")
        dbg["o2"] = nc.dram_tensor("dbg_o2", [128, 2, QL], F32, kind="ExternalOutput")

    with tile.TileContext(nc) as tc:
        with (
            tc.tile_pool(name="const", bufs=1) as cp,
            tc.tile_pool(name="work", bufs=1) as wp,
            tc.tile_pool(name="ppool", bufs=6) as ppool,
            tc.tile_pool(name="rbpool", bufs=2) as rbpool,
            tc.tile_pool(name="psw", bufs=2, space="PSUM") as pss,   # 2x2 banks
            tc.tile_pool(name="pav", bufs=4, space="PSUM") as pavp,  # 4x1 bank
            tc.tile_pool(name="dram", bufs=1, space="DRAM") as dp,
        ):
            # ---------------- input DMAs ----------------
            # Crit wave: ungated on sync/scalar/gpsimd. Everything else
            # streams from the sync queue (no compute there), ungated, in
            # consumption order, split into <=256KB pieces so several DMA
            # rings work per tensor. Mask tensors split by q-column: the
            # A-half feeds layer-1 pass A, the B-half only pass B.
            crit = []
            insts = {}

            def load(eng, dram, shape, dt=F32, gate=None, pool=cp, split=None,
                     defer=None):
                t = pool.tile(shape, dt, tag=f"c_{dram.name}", name=f"s_{dram.name}")
                pieces = []
                if split is None:
                    pieces.append((t[:], dram[:]))
                else:
                    n = shape[split]
                    h = n // 2
                    ix = tuple(slice(None) for _ in range(split))
                    pieces.append((t[ix + (slice(0, h),)], dram[ix + (slice(0, h),)]))
                    pieces.append((t[ix + (slice(h, n),)], dram[ix + (slice(h, n),)]))
                for dst, src_ in pieces:
                    inst = eng.dma_start(dst, src_)
                    insts[dram.name] = inst
                    if gate is not None:
                        for g in gate:
                            tile.add_dep_helper(inst.ins, g.ins, sync=True,
                                                reason="late input load")
                    else:
                        crit.append(inst)
                return t

            # crit: x blk0/1 -> K blk0/1 -> QK + q + V + mask0(A)
            wn1_s = load(nc.sync, wn[0], [128, 2, D], BF)
            wk1_s = load(nc.scalar, wk[0], [128, 2, D], BF)
            wq1_s = load(nc.gpsimd, wq[0], [128, 2, D], BF)
            wv1_s = load(nc.gpsimd, wv[0], [128, 2, D], BF)
            g_ch, ef_ch = [None] * 4, [None] * 4
            g_ch[0] = load(nc.sync, g_ch_d[0], [128, 2, 1024], BF, split=1)
            ef_ch[0] = load(nc.scalar, ef_ch_d[0], [128, 2, 1024], BF, split=1)
            g_loc_s = load(nc.scalar, g_loc, [128, 2, QL], BF)
            ef_loc_s = load(nc.gpsimd, ef_loc, [128, 2, QL], BF)
            bk1_s = load(nc.sync, bk[0], [128, 2])
            bq1_s = load(nc.scalar, bq[0], [128, 2])
            m_s = [None] * 8
            m_s[0] = load(nc.gpsimd, mask_d[0], [128, 4, QL], BF, split=1)
            m_s[1] = load(nc.sync, mask_d[1], [128, 4, QL], BF, split=1)

            # streaming wave on sync, consumption order
            for j in (1, 2, 3):
                g_ch[j] = load(nc.sync, g_ch_d[j], [128, 2, 1024], BF,
                               gate=[], split=1)
                m_s[2 * j] = load(nc.sync, mask_d[2 * j], [128, 4, QL], BF,
                                  gate=[], split=1)
                ef_ch[j] = load(nc.sync, ef_ch_d[j], [128, 2, 1024], BF,
                                gate=[], split=1)
                m_s[2 * j + 1] = load(nc.sync, mask_d[2 * j + 1],
                                      [128, 4, QL], BF, gate=[], split=1)
            wn2_s = load(nc.sync, wn[1], [128, 2, D], BF, gate=[])
            wk2_s = load(nc.sync, wk[1], [128, 2, D], BF, gate=[])
            wq2_s = load(nc.sync, wq[1], [128, 2, D], BF, gate=[])
            wv2_s = load(nc.sync, wv[1], [128, 2, D], BF, gate=[])
            wo2_s = load(nc.sync, wo[1], [64, H, D], BF, gate=[])
            wo1_s = load(nc.sync, wo[0], [64, H, D], BF, gate=[])
            bk2_s = load(nc.sync, bk[1], [128, 2], gate=[])
            bq2_s = load(nc.sync, bq[1], [128, 2], gate=[])
            bof1_s = load(nc.sync, bof[0], [128, 2], gate=[])
            bof2_s = load(nc.sync, bof[1], [128, 2], gate=[])
            g4 = [insts["w_n2"], insts["w_k2"]]
            wc1_s = load(nc.sync, wc1, [128, 2, D], BF, gate=g4)
            bc1_s = load(nc.sync, bc1, [128, 2], gate=g4)
            wc2_s = load(nc.sync, wc2, [128, 2, NCLS], BF, gate=g4)
            bc2_s = load(nc.sync, bc2, [NCLS, 1], gate=g4)
            id16_s = load(nc.sync, id16, [NCLS, NCLS], gate=g4)

            wn_s, wq_s, wk_s, wv_s = [wn1_s, wn2_s], [wq1_s, wq2_s], \
                [wk1_s, wk2_s], [wv1_s, wv2_s]
            wo_s, bk_s, bq_s = [wo1_s, wo2_s], [bk1_s, bk2_s], [bq1_s, bq2_s]
            bof_s = [bof1_s, bof2_s]

            mm = nc.tensor.matmul

            x_t = wp.tile([128, 2, E], BF, tag="x_t", name="x_t")
            k_t = wp.tile([128, 2, E], BF, tag="k_t", name="k_t")
            v_s = wp.tile([128, KT, H, HD + 2], BF, tag="v", name="v_s")
            nc.vector.memset(v_s[:, :, :, HD:HD + 2], 1.0)

            def proj_x_full(l, blk):
                """x^T (l=0: Wn.G + (ef+bn1); l=1: ctx2 = Wn2.G, no bias)."""
                bsl = slice(blk * 512, blk * 512 + 512)
                gch = g_ch[blk // 2]
                gsl = slice((blk % 2) * 512, (blk % 2) * 512 + 512)
                for dt in range(2):
                    dsl = slice(dt * 128, dt * 128 + 128)
                    ps = pss.tile([128, 512], F32, tag="s", name=f"psx{l}_{blk}_{dt}")
                    mm(ps[:], wn_s[l][:, 0, dsl], gch[:, 0, gsl],
                       start=True, stop=False)
                    mm(ps[:], wn_s[l][:, 1, dsl], gch[:, 1, gsl],
                       start=False, stop=True)
                    if l == 0:
                        nc.vector.tensor_tensor(x_t[:, dt, bsl], ps[:],
                                                ef_ch[blk // 2][:, dt, gsl],
                                                mybir.AluOpType.add)
                    elif blk % 2 == 0:
                        nc.scalar.copy(x_t[:, dt, bsl], ps[:])
                    else:
                        nc.vector.tensor_copy(x_t[:, dt, bsl], ps[:])

            def proj_k_full(l, blk):
                bsl = slice(blk * 512, blk * 512 + 512)
                for dt in range(2):
                    dsl = slice(dt * 128, dt * 128 + 128)
                    ps = pss.tile([128, 512], F32, tag="s", name=f"psk{l}_{blk}_{dt}")
                    mm(ps[:], wk_s[l][:, 0, dsl], x_t[:, 0, bsl],
                       start=True, stop=False)
                    mm(ps[:], wk_s[l][:, 1, dsl], x_t[:, 1, bsl],
                       start=False, stop=True)
                    nc.vector.tensor_scalar_add(k_t[:, dt, bsl], ps[:],
                                                bk_s[l][:, dt:dt + 1])

            def proj_v_full(l, i):
                """V rows for edge-tiles 2i, 2i+1 (bv folded into b_or)."""
                ps = pss.tile([128, 512], F32, tag="s", name=f"psv{l}_{i}")
                for half in range(2):
                    et = 2 * i + half
                    esl = slice(et * 128, et * 128 + 128)
                    osl = slice(half * 256, half * 256 + 256)
                    mm(ps[:, osl], x_t[:, 0, esl], wv_s[l][:, 0, :],
                       start=(half == 0), stop=False)
                    mm(ps[:, osl], x_t[:, 1, esl], wv_s[l][:, 1, :],
                       start=False, stop=(half == 1))
                for half in range(2):
                    nc.scalar.copy(
                        v_s[:, 2 * i + half, :, 0:HD],
                        ps[:, half * 256:half * 256 + 256].rearrange(
                            "p (h d) -> p h d", h=H))

            def xloc_q(l, xloc, resid):
                """local x (fp32 residual) + Q^T (bf16)."""
                xbf = wp.tile([128, 2, QL], BF, tag="xbf", name=f"xbf{l}")
                for dt in range(2):
                    dsl = slice(dt * 128, dt * 128 + 128)
                    ps = pss.tile([128, 512], F32, tag="s", name=f"psxl{l}_{dt}")
                    mm(ps[:], wn_s[l][:, 0, dsl], g_loc_s[:, 0, :],
                       start=True, stop=False)
                    mm(ps[:], wn_s[l][:, 1, dsl], g_loc_s[:, 1, :],
                       start=False, stop=True)
                    nc.vector.tensor_tensor(xloc[:, dt, :], ps[:],
                                            resid[:, dt, :],
                                            mybir.AluOpType.add)
                    nc.vector.tensor_copy(xbf[:, dt, :], xloc[:, dt, :])
                q_t = wp.tile([128, 2, QL], BF, tag="q_t", name=f"q_t{l}")
                for dt in range(2):
                    dsl = slice(dt * 128, dt * 128 + 128)
                    ps = pss.tile([128, 512], F32, tag="s", name=f"psq{l}_{dt}")
                    mm(ps[:], wq_s[l][:, 0, dsl], xbf[:, 0, :],
                       start=True, stop=False)
                    mm(ps[:], wq_s[l][:, 1, dsl], xbf[:, 1, :],
                       start=False, stop=True)
                    nc.vector.tensor_scalar_add(q_t[:, dt, :], ps[:],
                                                bq_s[l][:, dt:dt + 1])
                # fold the (bo + bv@Wo [+ bn2]) bias into the residual copy
                # now that q/k/v inputs (xbf) are already cast
                for dt in range(2):
                    nc.vector.tensor_scalar_add(xloc[:, dt, :], xloc[:, dt, :],
                                                bof_s[l][:, dt:dt + 1])
                return xbf, q_t

            def attn_pass(l, q_t, pav4, qsl, first, tag, steps, fills,
                          nkt):
                """Software-pipelined QK->exp->mask->attn@V over `steps`
                (a list of (pair, kt0)); fills[i] emits projection work just
                before step i. nkt=2 merges two kt per PSUM tile (one
                full-width EXP per pair of kt). attnV runs one step behind
                QK so PE never blocks on the exp->mask chain."""
                qn = qsl.stop - qsl.start
                pend = None  # pending attnV step

                def do_attnv(item):
                    pair, kt0, p_t = item
                    for j in range(nkt):
                        kt = kt0 + j
                        st = first and kt == 0
                        sp = kt == KT - 1
                        for hh in range(2):
                            mm(pav4[2 * pair + hh][0:HD + 1, qsl],
                               v_s[:, kt, 2 * pair + hh, 0:HD + 1],
                               p_t[:, hh, j, 0:qn] if nkt == 2
                               else p_t[:, hh, 0:qn],
                               start=st, stop=sp, skip_group_check=not first)

                for i, (pair, kt0) in enumerate(steps):
                    if i in fills:
                        fills[i]()
                    ps_t = pss.tile([128, 2, 2, 256] if nkt == 2
                                    else [128, 2, 512], F32, tag="s",
                                    name=f"s{tag}_{pair}_{kt0}")
                    p_t = ppool.tile([128, 2, 2, 256] if nkt == 2
                                     else [128, 2, 512], BF, tag="p")
                    if nkt == 2:
                        # layout [p, hh, kt, q]: bank hh holds both kt halves
                        # at one row-group position; kt=+0 starts (bank-wide
                        # clear), kt=+1 lands in its zeroed half.
                        for hh in range(2):
                            hsl = slice(hh * 64, hh * 64 + 64)
                            for j in range(2):
                                ksl = slice((kt0 + j) * 128,
                                            (kt0 + j) * 128 + 128)
                                mm(ps_t[:, hh, j, :], k_t[hsl, pair, ksl],
                                   q_t[hsl, pair, qsl],
                                   start=(j == 0), stop=(j == 1),
                                   tile_position=(hh * 64, 0))
                        nc.scalar.activation(
                            p_t[:].rearrange("p a b q -> p (a b q)"),
                            ps_t[:].rearrange("p a b q -> p (a b q)"),
                            AF.Exp)
                        msk = m_s[kt0 // 4][:, kt0 % 4:kt0 % 4 + 2, qsl]
                        nc.vector.tensor_mul(
                            p_t[:], p_t[:],
                            msk.unsqueeze(1).broadcast_to([128, 2, 2, qn]))
                    else:
                        ksl = slice(kt0 * 128, kt0 * 128 + 128)
                        mm(ps_t[:, 0, 0:qn], k_t[0:64, pair, ksl],
                           q_t[0:64, pair, qsl], start=True, stop=True,
                           tile_position=(0, 0))
                        mm(ps_t[:, 1, 0:qn], k_t[64:128, pair, ksl],
                           q_t[64:128, pair, qsl], start=True, stop=True,
                           tile_position=(64, 0))
                        nc.scalar.activation(p_t[:, :, 0:qn],
                                             ps_t[:, :, 0:qn], AF.Exp)
                        msk = m_s[kt0 // 4][:, kt0 % 4, qsl]
                        nc.vector.tensor_mul(
                            p_t[:, :, 0:qn], p_t[:, :, 0:qn],
                            msk.unsqueeze(1).broadcast_to([128, 2, qn]))
                    if pend is not None:
                        do_attnv(pend)
                    pend = (pair, kt0, p_t)
                do_attnv(pend)

            def finish_pass(l, pav4, xloc, oloc, qsl, tag):
                """1/denom, scale, out-projection (+rank-1 bias), residual."""
                qn = qsl.stop - qsl.start
                rcp_s = wp.tile([65, H, QL], F32, tag="rcp", name=f"rcp{tag}")
                rb0 = wp.tile([1, H, QL], F32, tag="rb0", name=f"rb0{tag}")
                for h in range(H):
                    nc.vector.reciprocal(rcp_s[64:65, h, qsl],
                                         pav4[h][64:65, qsl])
                nc.scalar.dma_start(rb0[0:1, :, qsl], rcp_s[64:65, :, qsl])
                aon = wp.tile([64, H, QL], BF, tag="aon", name=f"aon{tag}")
                for h in range(H):
                    rb = rbpool.tile([64, QL], F32, tag="rb", name=f"rb{tag}{h}")
                    nc.gpsimd.partition_broadcast(rb[0:64, qsl],
                                                  rb0[0:1, h, qsl])
                    nc.vector.tensor_mul(aon[0:64, h, qsl], pav4[h][0:64, qsl],
                                         rb[0:64, qsl])
                for dt in range(2):
                    dsl = slice(dt * 128, dt * 128 + 128)
                    ps = pss.tile([128, 512], F32, tag="s", name=f"pso{tag}_{dt}")
                    for h in range(H):
                        mm(ps[:, 0:qn], wo_s[l][0:HD, h, dsl], aon[0:HD, h, qsl],
                           start=(h == 0), stop=(h == H - 1))
                    nc.vector.tensor_tensor(oloc[:, dt, qsl], ps[:, 0:qn],
                                            xloc[:, dt, qsl],
                                            mybir.AluOpType.add)

            # ================= layer 1 =================
            xloc1 = wp.tile([128, 2, QL], F32, tag="xloc", name="xloc1")
            o1loc = wp.tile([128, 2, QL], F32, tag="oloc", name="o1loc")
            # prologue: first attention window needs x/K blk0-1, V i0-3, q1
            for blk in range(2):
                proj_x_full(0, blk)
                proj_k_full(0, blk)
            _, q1_t = xloc_q(0, xloc1, ef_loc_s)
            for i in range(4):
                proj_v_full(0, i)

            def blk1_fill_x(b):
                def f():
                    proj_x_full(0, b)
                return f

            def blk1_fill_kv(b):
                def f():
                    proj_k_full(0, b)
                    proj_v_full(0, 2 * b)
                    proj_v_full(0, 2 * b + 1)
                return f

            def ctx2_fill(b):
                def f():
                    proj_x_full(1, b)
                return f

            pav4 = [pavp.tile([128, QL], F32, tag="pav", name=f"pav{h}")
                    for h in range(H)]
            o1bf = wp.tile([128, 2, QL], BF, tag="o1bf", name="o1bf")
            cc_in = [dp.tile([128, 2, QH], BF, name=f"cc_in{p}") for p in range(2)]
            cc_out = [dp.tile([N_CORES, 128, 2, QH], BF, name=f"cc_out{p}")
                      for p in range(2)]
            # gathered o1 organized [d-part, dt, core, half, 256]
            o1g = wp.tile([128, 2, N_CORES, 2, QH], BF, tag="o1g", name="o1g")
            o1g_rest = []

            def o1g_drain(p):
                # cores 4..7 pieces, emitted late enough that their CC-gated
                # triggers never block the compute queues
                for pp, cco in [x for x in o1g_rest if x[0] == p]:
                    for c in (4, 5):
                        nc.scalar.dma_start(o1g[:, :, c, pp, :], cco[c])
                    for c in (6, 7):
                        nc.gpsimd.dma_start(o1g[:, :, c, pp, :], cco[c])

            steps1 = [(pair, k) for pair in range(2) for k in range(0, KT, 2)]
            # pass A: stream remaining L1 blocks in (one per odd step, just
            # ahead of the window that consumes them); pass B: build ctx2
            # (layer-2 Wn.G, overwrites x_t behind L1's last readers)
            fillsA = {}
            for b in range(2, 8):
                fillsA[2 * b - 4] = blk1_fill_x(b)
                fillsA[2 * b - 3] = blk1_fill_kv(b)
            fillsB = {2 * b + 1: ctx2_fill(b) for b in range(8)}
            for p, qsl in enumerate((slice(0, QH), slice(QH, QL))):
                attn_pass(0, q1_t, pav4, qsl, first=(p == 0), tag=f"a{p}",
                          steps=steps1, fills=fillsA if p == 0 else fillsB,
                          nkt=2)
                finish_pass(0, pav4, xloc1, o1loc, qsl, tag=f"f{p}")
                for dt in range(2):
                    nc.gpsimd.tensor_copy(o1bf[:, dt, qsl], o1loc[:, dt, qsl])
                nc.gpsimd.dma_start(cc_in[p][:], o1bf[:, :, qsl])
                nc.gpsimd.collective_compute(
                    "AllGather",
                    mybir.AluOpType.bypass,
                    replica_groups=[list(range(N_CORES))],
                    ins=[cc_in[p][:].opt()],
                    outs=[cc_out[p][:].opt()],
                )
                for c in range(4):
                    nc.sync.dma_start(o1g[:, :, c, p, :], cc_out[p][c])
                o1g_rest.append((p, cc_out[p]))

            if DEBUG:
                nc.sync.dma_start(dbg["x_t"][:], x_t[:])
                nc.sync.dma_start(dbg["k_t"][:], k_t[:])
                nc.sync.dma_start(dbg["q_t"][:], q1_t[:])
                nc.sync.dma_start(dbg["o1"][:], o1loc[:])

            # ============ layer 2 ============
            # Edge split by gather half: core b's edges [b*512, b*512+512);
            # cols 0:256 (kt 4b,4b+1) need only gather-A, cols 256:512
            # (kt 4b+2,4b+3) need gather-B. Attention runs all A-half kt
            # tiles first, so it starts as soon as gather-A lands and
            # gather-B hides under the A phase.
            o1g_drain(0)
            xloc2 = wp.tile([128, 2, QL], F32, tag="xloc", name="xloc2")
            o2loc = wp.tile([128, 2, QL], F32, tag="oloc", name="o2loc")
            _, q2_t = xloc_q(1, xloc2, o1loc)

            def half_fill(b, p):
                def f():
                    hsl = slice(b * 512 + p * QH, b * 512 + (p + 1) * QH)
                    for dt in range(2):
                        nc.gpsimd.tensor_add(x_t[:, dt, hsl], x_t[:, dt, hsl],
                                             o1g[:, dt, b, p, :])
                    for dt in range(2):
                        dsl = slice(dt * 128, dt * 128 + 128)
                        ps = pss.tile([128, 256], F32, tag="s",
                                      name=f"psk2_{b}_{p}_{dt}")
                        mm(ps[:], wk_s[1][:, 0, dsl], x_t[:, 0, hsl],
                           start=True, stop=False)
                        mm(ps[:], wk_s[1][:, 1, dsl], x_t[:, 1, hsl],
                           start=False, stop=True)
                        nc.vector.tensor_scalar_add(k_t[:, dt, hsl], ps[:],
                                                     bk_s[1][:, dt:dt + 1])
                    proj_v_full(1, 2 * b + p)
                return f

            kA = [k for b in range(8) for k in (4 * b, 4 * b + 1)]
            kB = [k for b in range(8) for k in (4 * b + 2, 4 * b + 3)]
            steps2 = ([(0, k) for k in kA] + [(1, k) for k in kA] +
                      [(0, k) for k in kB] + [(1, k) for k in kB])
            fills2 = {}
            for b in range(2, 8):
                fills2[2 * b - 3] = half_fill(b, 0)       # A blocks 2..7
            fills2[23] = lambda: o1g_drain(1)
            for b in range(8):
                fills2[25 + 2 * b] = half_fill(b, 1)      # B blocks 0..7
            # prologue: A-halves of blocks 0,1
            half_fill(0, 0)()
            half_fill(1, 0)()

            pav4b = [pavp.tile([128, QL], F32, tag="pav", name=f"pav2_{h}")
                     for h in range(H)]
            attn_pass(1, q2_t, pav4b, slice(0, QL), first=True, tag="b",
                      steps=steps2, fills=fills2, nkt=1)
            finish_pass(1, pav4b, xloc2, o2loc, slice(0, QL), tag="fb")

            if DEBUG:
                nc.sync.dma_start(dbg["x2"][:], x_t[:])
                nc.sync.dma_start(dbg["o2"][:], o2loc[:])

            # ============ classifier ============
            o2bf = wp.tile([128, 2, QL], BF, tag="o2bf", name="o2bf")
            for dt in range(2):
                nc.vector.tensor_copy(o2bf[:, dt, :], o2loc[:, dt, :])
            h_s = wp.tile([128, 2, QL], BF, tag="h", name="h_s")
            for dt in range(2):
                dsl = slice(dt * 128, dt * 128 + 128)
                ps = pss.tile([128, 512], F32, tag="s", name=f"psc{dt}")
                mm(ps[:], wc1_s[:, 0, dsl], o2bf[:, 0, :], start=True, stop=False)
                mm(ps[:], wc1_s[:, 1, dsl], o2bf[:, 1, :], start=False, stop=True)
                nc.scalar.activation(h_s[:, dt, :], ps[:], AF.Gelu,
                                     bias=bc1_s[:, dt:dt + 1])
            ps_l = pss.tile([128, 512], F32, tag="s", name="ps_l")
            mm(ps_l[0:NCLS, :], wc2_s[:, 0, :], h_s[:, 0, :], start=True, stop=False)
            mm(ps_l[0:NCLS, :], wc2_s[:, 1, :], h_s[:, 1, :], start=False, stop=True)
            lg = wp.tile([NCLS, QL], F32, tag="lg", name="lg")
            nc.scalar.activation(lg[:], ps_l[0:NCLS, :], AF.Identity,
                                 bias=bc2_s[:, 0:1])
            out_s = wp.tile([128, 4, NCLS], F32, tag="outs", name="out_s")
            for qt in range(4):
                ps = pss.tile([128, 512], F32, tag="s", name=f"pst{qt}")
                nc.tensor.transpose(ps[0:128, 0:NCLS],
                                    lg[0:NCLS, qt * 128:qt * 128 + 128],
                                    id16_s[:, :])
                nc.vector.tensor_copy(out_s[:, qt, :], ps[0:128, 0:NCLS])
            nc.sync.dma_start(out[:].rearrange("(qt p) j -> p qt j", p=128), out_s[:])

    nc.compile()
    return nc


# --------------------------------------------------------------------------
# host-side data prep
# --------------------------------------------------------------------------

def _tiles_T(a):
    """[E2, D2] array -> transposed tile layout [128, D2//128, E2]."""
    d2 = a.shape[1]
    return np.ascontiguousarray(
        a.T.reshape(d2 // 128, 128, a.shape[0]).transpose(1, 0, 2))


def _wtile(w):
    """[G, D] weight -> [128, G//128, D] (lhsT tiles, partition=contraction)."""
    g, d = w.shape
    return np.ascontiguousarray(w.reshape(g // 128, 128, d).transpose(1, 0, 2))


def _btile(b):
    return np.ascontiguousarray(b.reshape(-1, 128).T)  # [128, 2]


def prep_in_maps(inputs):
    f32 = np.float32
    nf = np.asarray(inputs["node_features"], f32)
    ef = np.asarray(inputs["edge_features"], f32)
    ei = np.asarray(inputs["edge_index"], np.int32)
    src, dst = ei[0], ei[1]

    G = np.concatenate([nf[src], nf[dst]], axis=1)            # [E, 256]
    g_t = _tiles_T(G).astype(BF16)                             # [128, 2, E]
    ef_t_f = _tiles_T(ef)                                      # [128, 2, E] f32
    bn1_w = np.asarray(inputs["a1_bn"], f32)
    bn1_col = bn1_w.reshape(2, 128).T                          # [128, 2]
    efb_t = (ef_t_f + bn1_col[:, :, None]).astype(BF16)        # ef + bn1

    adj = ((src[:, None] == src[None, :]) | (src[:, None] == dst[None, :]) |
           (dst[:, None] == src[None, :]) | (dst[:, None] == dst[None, :]))
    adj_t = adj.reshape(KT, 128, E).transpose(1, 0, 2)         # [128, KT, E]

    com = {}
    for j in range(4):
        sl = slice(j * 1024, (j + 1) * 1024)
        com[f"g_ch{j}"] = np.ascontiguousarray(g_t[:, :, sl])
        com[f"ef_ch{j}"] = np.ascontiguousarray(efb_t[:, :, sl])
    bn2_w = np.asarray(inputs["a2_bn"], f32)
    for l, pre in ((1, "a1"), (2, "a2")):
        com[f"w_n{l}"] = _wtile(np.asarray(inputs[f"{pre}_Wn"], f32)).astype(BF16)
        com[f"w_q{l}"] = _wtile(np.asarray(inputs[f"{pre}_Wq"], f32) * SQ).astype(BF16)
        com[f"w_k{l}"] = _wtile(np.asarray(inputs[f"{pre}_Wk"], f32)).astype(BF16)
        com[f"w_v{l}"] = _wtile(np.asarray(inputs[f"{pre}_Wv"], f32)).astype(BF16)
        Wo = np.asarray(inputs[f"{pre}_Wo"], f32)
        com[f"w_o{l}"] = np.ascontiguousarray(
            Wo.reshape(H, HD, D).transpose(1, 0, 2)).astype(BF16)
        com[f"b_q{l}"] = _btile(np.asarray(inputs[f"{pre}_bq"], f32) * SQ)
        com[f"b_k{l}"] = _btile(np.asarray(inputs[f"{pre}_bk"], f32))
        # attention rows sum to 1 => value bias passes through attn@v;
        # fold into out-proj bias; bn2 rides on o1 so remote cores get it.
        bo_eff = (np.asarray(inputs[f"{pre}_bo"], f32) +
                  np.asarray(inputs[f"{pre}_bv"], f32) @ Wo)
        if l == 1:
            bo_eff = bo_eff + bn2_w
        com[f"b_or{l}"] = np.ascontiguousarray(
            bo_eff.reshape(1, 2, 128)).astype(BF16)
        com[f"b_of{l}"] = _btile(bo_eff)
    com["w_c1"] = _wtile(np.asarray(inputs["cls_W1"], f32)).astype(BF16)
    com["b_c1"] = _btile(np.asarray(inputs["cls_b1"], f32))
    com["w_c2"] = _wtile(np.asarray(inputs["cls_W2"], f32)).astype(BF16)
    com["b_c2"] = np.asarray(inputs["cls_b2"], f32).reshape(NCLS, 1)
    com["id16"] = np.eye(NCLS, dtype=f32)

    in_maps = []
    for c in range(N_CORES):
        q = slice(c * QL, (c + 1) * QL)
        m = dict(com)
        m["ef_loc"] = np.ascontiguousarray(
            ef_t_f[:, :, q] + bn1_col[:, :, None]).astype(BF16)
        m["g_loc"] = np.ascontiguousarray(g_t[:, :, q])
        for j in range(8):
            m[f"mask{j}"] = np.ascontiguousarray(
                adj_t[:, j * 4:(j + 1) * 4, q]).astype(BF16)
        in_maps.append(m)
    return in_maps


_NC_CACHE = None


def kernel(**inputs) -> np.ndarray:
    global _NC_CACHE
    in_maps = prep_in_maps(inputs)
    if _NC_CACHE is None:
        _NC_CACHE = build_nc()
    res = run_bass_kernel_spmd(_NC_CACHE, in_maps, core_ids=list(range(N_CORES)))
    return np.concatenate([res.results[c]["out"] for c in range(N_CORES)], axis=0)
